# revision 35
# baseline (speedup 1.0000x reference)
"""Trainium2 Bass kernel for BatchGATConv (GAT message passing).

Strategy (8 NeuronCores, SPMD):
  - Edges are partitioned by destination-node range: core c owns dst nodes
    [c*2500, (c+1)*2500). Host sorts edges by dst and packs them, per
    128-node "node tile", into 128-edge chunks (padded with dstl=-1 slots).
  - Each core (replicated work) projects all node features ft = feat @ W and
    per-node logits el/er (attn vectors folded into the weight matrix on the
    host), writing a gather table row per (node, batch):
    g[2n+b] = [ft(n,b) 256 | el(n,b) 4 | er(n,b) 4].
  - Per 128-edge chunk: one indirect-DMA gather of the row PAIR g[2*src]
    (528 floats), edge logits e = leaky(el_src + er_dst) with er_dst
    selected via a one-hot matmul, ex = exp(e) (no max-subtraction needed:
    logits are O(5), exp is safe in fp32 and the softmax is mathematically
    identical), messages m = ft*ex, then a one-hot segment-sum matmul
    accumulating numerator and denominator in PSUM across the tile's chunks.
    Per node tile: out = leaky(num/denom), written contiguously.
  - Matmuls run as float32r (fp32 bits, single-pass PE streaming mode).
"""

import numpy as np

try:
    import concourse.bass as bass
except ImportError:  # pragma: no cover
    import sys

    sys.path.insert(0, "/opt/trn_rl_repo")
    import concourse.bass as bass

import concourse.bacc as bacc
import concourse.mybir as mybir
import concourse.tile as tile
from concourse.bass_utils import run_bass_kernel_spmd

P = 128
F32 = mybir.dt.float32
F32R = mybir.dt.float32r
I32 = mybir.dt.int32

# problem constants
N, B, DIN, H, D, E = 20000, 2, 128, 4, 64, 320000
NEG = 0.2
NCORES = 8
HB = H * B  # 8 logits per node/edge
HD = H * D  # 256 projected feats per (n, b)
FT = B * H * D  # 512 projected feats per node
WC = HD + 2 * H  # 264 = [W | W.attn_l | W.attn_r] columns; also g-row width
GW = 2 * WC  # 528 = gathered row-pair width

USE_F32R = False
# dtype for matmul operands: f32r streams through the PE in a single pass
# (vs 2 half-speed passes for plain fp32). The BIR verifier requires every
# f32r-matmul operand to be produced with dtype f32r, so the operand tiles
# (and the DRAM tensors DMA'd straight into them) are declared f32r; numpy
# bits are identical fp32 either way.
MMDT = F32R if USE_F32R else F32


def _mm(ap):
    return ap


def _host_prep(src, dst, n_nodes, n_cores):
    """Sort edges by dst; pack per (core, node-tile) into 128-edge chunks.

    Returns (K, idx_T, dstl_T, nid_T):
      K: per-node-tile chunk count (shared across cores; program structure)
      idx_T[c]:  [P, sum(K)] int32, gather row (= 2*src) per chunk slot
      dstl_T[c]: [P, sum(K)] float32, dst-local index in [0,128) or -1 pad
      nid_T[c]:  [P, nt] int32, gather row (= 2*node) per tile partition
    """
    npc = n_nodes // n_cores
    nt = (npc + P - 1) // P
    order = np.argsort(dst, kind="stable")
    ss = np.ascontiguousarray(src[order]).astype(np.int64)
    ds = np.ascontiguousarray(dst[order]).astype(np.int64)

    lows = np.array(
        [c * npc + t * P for c in range(n_cores) for t in range(nt + 1)],
        dtype=np.int64,
    )
    lows = np.minimum(lows, n_nodes)
    bounds = np.searchsorted(ds, lows).reshape(n_cores, nt + 1)
    cnts = bounds[:, 1:] - bounds[:, :-1]  # [n_cores, nt]

    K = np.maximum(1, -(-cnts.max(axis=0) // P)).astype(np.int64)  # per tile
    SK = int(K.sum())
    offs = np.concatenate([[0], np.cumsum(K)[:-1]])

    idx_all = np.zeros((n_cores, SK, P), np.int32)
    dstl_all = np.full((n_cores, SK, P), -1.0, np.float32)
    for c in range(n_cores):
        for t in range(nt):
            e0 = bounds[c, t]
            cnt = int(cnts[c, t])
            if cnt == 0:
                continue
            s = np.arange(cnt)
            rows = offs[t] + s // P
            cols = s % P
            idx_all[c, rows, cols] = 2 * ss[e0 : e0 + cnt]
            dstl_all[c, rows, cols] = (ds[e0 : e0 + cnt] - (c * npc + t * P)).astype(
                np.float32
            )

    nid_all = np.zeros((n_cores, nt, P), np.int32)
    base = np.arange(P)
    for c in range(n_cores):
        for t in range(nt):
            nid_all[c, t] = 2 * np.minimum(c * npc + t * P + base, n_nodes - 1)

    idx_T = [np.ascontiguousarray(idx_all[c].T) for c in range(n_cores)]
    dstl_T = [np.ascontiguousarray(dstl_all[c].T) for c in range(n_cores)]
    nid_T = [np.ascontiguousarray(nid_all[c].T) for c in range(n_cores)]
    # transposed one-hot selector per chunk: st[chunk, dst_local, slot] = 1
    st_T = []
    for c in range(n_cores):
        st = np.zeros((SK, P, P), np.float32)
        ch, dl, sl = np.nonzero(dstl_all[c][:, None, :] == np.arange(P)[None, :, None])
        st[ch, dl, sl] = 1.0
        st_T.append(np.ascontiguousarray(st.reshape(SK * P, P)))
    return list(map(int, K)), idx_T, dstl_T, nid_T, st_T


def _build(n_nodes, npc, K):
    """Build the SPMD Bass program (identical for all cores)."""
    R = n_nodes * B
    RT = (R + P - 1) // P
    nt = len(K)
    SK = sum(K)

    nc = bacc.Bacc(trn_type="TRN2", num_swdge_queues=2)
    featT = nc.dram_tensor("featT", [DIN, R], MMDT, kind="ExternalInput")
    wmat = nc.dram_tensor("wmat", [DIN, WC], MMDT, kind="ExternalInput")
    idxd = nc.dram_tensor("idx", [P, SK], I32, kind="ExternalInput")
    dstld = nc.dram_tensor("dstl", [P, SK], F32, kind="ExternalInput")
    nidd = nc.dram_tensor("nid", [P, nt], I32, kind="ExternalInput")
    sttd = nc.dram_tensor("stt", [SK * P, P], MMDT, kind="ExternalInput")
    outd = nc.dram_tensor("out", [npc, FT], F32, kind="ExternalOutput")
    g = nc.dram_tensor("gtab", [R, WC], F32)

    with tile.TileContext(nc) as tc:
        with (
            tc.tile_pool(name="const", bufs=1) as cp,
            tc.tile_pool(name="proj", bufs=8) as pp,
            tc.tile_pool(name="projps", bufs=2, space="PSUM") as ppp,
            tc.tile_pool(name="agg", bufs=12) as ag,
            tc.tile_pool(name="aggo", bufs=4) as og,
            tc.tile_pool(name="accps", bufs=2, space="PSUM") as psp,
            tc.tile_pool(name="smallps", bufs=2, space="PSUM") as psp1,
        ):
            # Resident constants. Matmul operands are routed through DVE
            # copies so matmul waits merge into a single DVE semaphore (the
            # fp32/f32r Matmult ISA struct has one sync-wait slot; Bacc's
            # event-semaphore pass splits the rest, but fewer is faster).
            w_sb0 = cp.tile([DIN, WC], MMDT)
            nc.sync.dma_start(w_sb0[:], wmat[:])
            w_sb = cp.tile([DIN, WC], MMDT)
            nc.vector.tensor_copy(w_sb[:], w_sb0[:])
            iota_i = cp.tile([P, P], I32)
            nc.gpsimd.iota(iota_i[:], pattern=[[1, P]], base=0, channel_multiplier=0)
            iota_f = cp.tile([P, P], F32)
            nc.vector.tensor_copy(iota_f[:], iota_i[:])
            idx_res = cp.tile([P, SK], I32)
            nc.sync.dma_start(idx_res[:], idxd[:])
            dstl_res = cp.tile([P, SK], F32)
            nc.sync.dma_start(dstl_res[:], dstld[:])
            nid_res = cp.tile([P, nt], I32)
            nc.sync.dma_start(nid_res[:], nidd[:])
            zero0 = cp.tile([P, FT], F32)
            nc.gpsimd.memset(zero0[:], 0.0)
            zero_sb = cp.tile([P, FT], F32)
            nc.vector.tensor_copy(zero_sb[:], zero0[:])

            # ---- projection: g[2n+b] = [ft(n,b) | el(n,b) | er(n,b)] ----
            for it in range(RT):
                r0 = it * P
                rows = min(P, R - r0)
                qs = (nc.sync, nc.scalar, nc.gpsimd)
                ftT = pp.tile([DIN, rows], MMDT, tag="ftT")
                qs[it % 3].dma_start(ftT[:], featT[:, r0 : r0 + rows])
                po = ppp.tile([rows, WC], F32, tag="po")
                nc.tensor.matmul(
                    po[:], lhsT=_mm(ftT[:]), rhs=_mm(w_sb[:]), start=True, stop=True
                )
                pout = pp.tile([rows, WC], F32, tag="pout")
                nc.vector.tensor_copy(pout[:], po[:])
                qs[(it + 1) % 3].dma_start(out=g[r0 : r0 + rows, :], in_=pout[:])

            # ---- aggregation: per node tile, segment softmax + weighted sum ----
            off = 0
            for t in range(nt):
                tn = min(P, npc - t * P)
                # er for the tile's own nodes: indirect gather reads
                # CONTIGUOUS source bytes, so fetch the 268-element span from
                # er(n,b0) at col 260 of row 2n through er(n,b1) at the end of
                # row 2n+1; er(b0) lands at flat cols 0:4, er(b1) at 264:268.
                er_t0 = og.tile([P, B, WC], F32, tag="er_t0")
                nc.gpsimd.indirect_dma_start(
                    out=er_t0[:].rearrange("p b c -> p (b c)")[:, 0 : WC + H],
                    out_offset=None,
                    in_=g[:],
                    in_offset=bass.IndirectOffsetOnAxis(
                        ap=nid_res[:, t : t + 1], axis=0
                    ),
                    element_offset=HD + H,
                )
                er_t = og.tile([P, HB], MMDT, tag="er_t")
                nc.vector.tensor_copy(
                    er_t[:].rearrange("p (b h) -> p b h", b=B),
                    er_t0[:, :, 0:H],
                )
                acc_a = psp.tile([P, WC], F32, tag="acca")
                acc_b = psp.tile([P, HD], F32, tag="accb")
                # zero-clear via DVE so PSUM bank-WAW/WAR waits stay off the
                # accumulating matmuls; adding onto DVE-written zeros is exact
                # whether or not the write cleared the has_written bits.
                nc.vector.tensor_copy(acc_a[:], zero_sb[:, :WC])
                nc.vector.tensor_copy(acc_b[:], zero_sb[:, :HD])
                for k in range(K[t]):
                    col = off + k
                    gt = ag.tile([P, GW], F32, tag="gt")
                    nc.gpsimd.indirect_dma_start(
                        out=gt[:],
                        out_offset=None,
                        in_=g[:],
                        in_offset=bass.IndirectOffsetOnAxis(
                            ap=idx_res[:, col : col + 1], axis=0
                        ),
                    )
                    gt3 = gt[:].rearrange("p (b c) -> p b c", b=B)
                    S = ag.tile([P, P], MMDT, tag="S")
                    nc.vector.tensor_scalar(
                        out=S[:],
                        in0=iota_f[:],
                        scalar1=dstl_res[:, col : col + 1],
                        scalar2=None,
                        op0=mybir.AluOpType.is_equal,
                    )
                    S_T = ag.tile([P, P], MMDT, tag="ST")
                    nc.sync.dma_start(S_T[:], sttd[col * P : (col + 1) * P, :])
                    eep = psp1.tile([P, HB], F32, tag="eep")
                    nc.tensor.matmul(
                        eep[:], lhsT=_mm(S_T[:]), rhs=_mm(er_t[:]), start=True, stop=True
                    )
                    lg = ag.tile([P, HB], F32, tag="lg")
                    nc.vector.tensor_add(
                        lg[:].rearrange("p (b h) -> p b h", b=B),
                        gt3[:, :, HD : HD + H],
                        eep[:].rearrange("p (b h) -> p b h", b=B),
                    )
                    l1 = ag.tile([P, HB], F32, tag="l1")
                    nc.vector.scalar_tensor_tensor(
                        out=l1[:],
                        in0=lg[:],
                        scalar=NEG,
                        in1=lg[:],
                        op0=mybir.AluOpType.mult,
                        op1=mybir.AluOpType.max,
                    )
                    # m_ext = [m(b0) 256 | exs 8 | m(b1) 256]: the first 264
                    # columns and last 256 columns feed two >=256-wide f32r
                    # matmuls, folding the denominator into the first bank.
                    m_ext = ag.tile([P, 2, WC], MMDT, tag="m")
                    nc.scalar.activation(
                        m_ext[:, 0, HD:HD + HB], l1[:],
                        mybir.ActivationFunctionType.Exp,
                    )
                    exs_v = m_ext[:, 0, HD:HD + HB]
                    nc.vector.tensor_tensor(
                        out=m_ext[:, :, 0:HD].rearrange("p b (h d) -> p b h d", d=D),
                        in0=gt3[:, :, 0:HD].rearrange("p b (h d) -> p b h d", d=D),
                        in1=exs_v.rearrange("p (b h) -> p b h", b=B)[:, :, :, None]
                        .to_broadcast([P, B, H, D]),
                        op=mybir.AluOpType.mult,
                    )
                    nc.tensor.matmul(
                        acc_a[:],
                        lhsT=_mm(S[:]),
                        rhs=_mm(m_ext[:].rearrange("p b c -> p (b c)")[:, 0:WC]),
                        start=False,
                        stop=(k == K[t] - 1),
                        skip_group_check=True,
                    )
                    nc.tensor.matmul(
                        acc_b[:],
                        lhsT=_mm(S[:]),
                        rhs=_mm(m_ext[:, 1, 0:HD]),
                        start=False,
                        stop=(k == K[t] - 1),
                        skip_group_check=True,
                    )
                off += K[t]
                dsum = og.tile([P, HB], F32, tag="dsum")
                nc.vector.tensor_scalar_add(dsum[:], acc_a[:, HD:HD + HB], 1e-30)
                rcp = og.tile([P, HB, 1], F32, tag="rcp")
                nc.vector.reciprocal(rcp[:, :, 0], dsum[:])
                o1 = og.tile([P, HB, D], F32, tag="o1")
                nc.vector.tensor_tensor(
                    out=o1[:, 0:H, :],
                    in0=acc_a[:, 0:HD].rearrange("p (h d) -> p h d", d=D),
                    in1=rcp[:, 0:H].to_broadcast([P, H, D]),
                    op=mybir.AluOpType.mult,
                )
                nc.vector.tensor_tensor(
                    out=o1[:, H:HB, :],
                    in0=acc_b[:].rearrange("p (h d) -> p h d", d=D),
                    in1=rcp[:, H:HB].to_broadcast([P, H, D]),
                    op=mybir.AluOpType.mult,
                )
                o3 = og.tile([P, FT], F32, tag="o3")
                nc.vector.scalar_tensor_tensor(
                    out=o3[:].rearrange("p (h d) -> p h d", d=D),
                    in0=o1[:],
                    scalar=NEG,
                    in1=o1[:],
                    op0=mybir.AluOpType.mult,
                    op1=mybir.AluOpType.max,
                )
                nc.sync.dma_start(out=outd[t * P : t * P + tn, :], in_=o3[:tn, :])

    nc.compile()
    _check_matmul_waits(nc)
    return nc


def _check_matmul_waits(nc):
    """fp32/f32r Matmult has a single ISA sync-wait slot; walrus codegen
    hard-fails on more. Catch it at build time."""
    bad = []
    for bb in nc.main_func.blocks:
        for ins in bb.instructions:
            if type(ins).__name__ == "InstMatmult":
                si = ins.sync_info
                nw = len(si.on_wait) if si is not None and si.on_wait else 0
                if nw > 1:
                    bad.append((ins.name, [w.ant_name for w in si.on_wait]))
    if bad:
        raise RuntimeError(f"matmuls with >1 sync wait: {bad[:10]} (n={len(bad)})")


def _make_inputs(feat, W, attn_l, attn_r, src, dst, n_nodes, n_cores):
    feat = np.asarray(feat, dtype=np.float32)
    W = np.asarray(W, dtype=np.float32)
    attn_l = np.asarray(attn_l, dtype=np.float32)
    attn_r = np.asarray(attn_r, dtype=np.float32)
    src = np.asarray(src)
    dst = np.asarray(dst)

    featT = np.ascontiguousarray(feat.reshape(n_nodes * B, DIN).T)
    Wl = (W.reshape(DIN, H, D) * attn_l[None]).sum(-1).astype(np.float32)
    Wr = (W.reshape(DIN, H, D) * attn_r[None]).sum(-1).astype(np.float32)
    wmat = np.ascontiguousarray(np.concatenate([W, Wl, Wr], axis=1))

    K, idx_T, dstl_T, nid_T, st_T = _host_prep(src, dst, n_nodes, n_cores)
    in_maps = [
        {
            "featT": featT,
            "wmat": wmat,
            "idx": idx_T[c],
            "dstl": dstl_T[c],
            "nid": nid_T[c],
            "stt": st_T[c],
        }
        for c in range(n_cores)
    ]
    return K, in_maps


_CACHE = {}


def kernel(feat, W, attn_l, attn_r, src, dst):
    K, in_maps = _make_inputs(feat, W, attn_l, attn_r, src, dst, N, NCORES)
    key = tuple(K)
    if key not in _CACHE:
        _CACHE[key] = _build(N, N // NCORES, K)
    nc = _CACHE[key]
    res = run_bass_kernel_spmd(nc, in_maps, list(range(NCORES))).results
    out = np.concatenate([res[c]["out"] for c in range(NCORES)], axis=0)
    return np.ascontiguousarray(out.reshape(N, B, H, D))


if __name__ == "__main__":
    rng = np.random.default_rng(0)
    feat = rng.standard_normal((N, B, DIN), dtype=np.float32)
    W = rng.standard_normal((DIN, H * D), dtype=np.float32) / np.sqrt(DIN)
    al = rng.standard_normal((H, D), dtype=np.float32) * 0.1
    ar = rng.standard_normal((H, D), dtype=np.float32) * 0.1
    src = rng.integers(0, N, E).astype(np.int32)
    dst = rng.integers(0, N, E).astype(np.int32)
    out = kernel(feat=feat, W=W, attn_l=al, attn_r=ar, src=src, dst=dst)
    print(out.shape, out.dtype, np.abs(out).mean())
